# revision 17
# baseline (speedup 1.0000x reference)
"""LSTM encoder (nn_Encoder_83528523972689) on 8 TRN2 NeuronCores.

Strategy: tensor-parallel split of the 4H gate dimension across 8 cores.
Each core owns a 128-wide slice of H for every gate (i/f/o/g). Per step it
computes its gates.T [512, 32] slice (W_hh slice stationary in the PE,
h.T moving), updates its c/h slices, and broadcasts its h.T chunk
[128, 32] to all 8 cores via remote_dma_broadcast (direct SBUF->SBUF,
relative XOR routing). x_proj = emb[x] @ W_ih.T + b is precomputed in a
parallel phase (embedding rows gathered transposed via dma_gather).

Everything is raw Bass (no Tile), fully unrolled over the 512 steps,
monotone semaphore thresholds, with a startup all-core barrier.
"""

import sys

sys.path.insert(0, "/opt/trn_rl_repo")

import numpy as np
import ml_dtypes

import concourse.bass as bass
import concourse.mybir as mybir
from concourse import library_config
from concourse.library_overlay import lower_extended_insts

F32 = mybir.dt.float32
BF16 = mybir.dt.bfloat16
I16 = mybir.dt.int16
AF = mybir.ActivationFunctionType

N_CORES = 8
B = 32  # batch
T = 512  # seq len
E = 512  # embedding dim
H = 1024  # hidden
G = 512  # gate rows per core (4 gates x 128)
VOCAB = 32000

# tokens processed per phase-1 gather chunk
TOK = B * T  # 16384
CHUNK = 2048
N_CHUNK = TOK // CHUNK  # 8
TQ = 512  # tokens per phase-1 matmul group
N_Q = TOK // TQ  # 32 groups

_cache = {}


def build_nc(t_steps=T):
    TT = t_steps
    TOKL = B * TT
    CH = min(512, TOKL)
    N_CH = TOKL // CH
    N_QL = TOKL // TQ
    nc = bass.Bass(num_devices=N_CORES)

    # ---------------- I/O ----------------
    emb = nc.declare_dram_parameter("emb", [VOCAB, E], BF16, isOutput=False)
    xidx = nc.declare_dram_parameter("xidx", [128, TOKL // 16], I16, isOutput=False)
    wih = nc.declare_dram_parameter("wih", [128, 16 * 128], BF16, isOutput=False)
    whh = nc.declare_dram_parameter("whh", [128, 32 * 128], BF16, isOutput=False)
    bias = nc.declare_dram_parameter("bias", [128, 4], F32, isOutput=False)
    hs_out = nc.declare_dram_parameter("hs", [TT, 128, B], BF16, isOutput=True)
    hf_out = nc.declare_dram_parameter("hf", [128, B], BF16, isOutput=True)
    cf_out = nc.declare_dram_parameter("cf", [128, B], F32, isOutput=True)

    # internal HBM scratch: x_proj.T slice, layout [4 g][128 p][T t][B b] f32
    xproj = nc.dram_tensor("xproj", [4, 128, TT, B], F32)

    # ---------------- SBUF ----------------
    xidx_sb = nc.alloc_sbuf_tensor("xidx_sb", [128, TOKL // 16], I16)
    wih_sb = nc.alloc_sbuf_tensor("wih_sb", [128, 16 * 128], BF16)
    whh_sb = nc.alloc_sbuf_tensor("whh_sb", [128, 32 * 128], BF16)
    bias_sb = nc.alloc_sbuf_tensor("bias_sb", [128, 4], F32)
    embT = [
        nc.alloc_sbuf_tensor(f"embT{i}", [128, 4, CH], BF16) for i in range(2)
    ]
    xp_st = [
        nc.alloc_sbuf_tensor(f"xp_st{i}", [128, 4 * TQ], F32) for i in range(2)
    ]
    # phase 2
    h_all = [nc.alloc_sbuf_tensor(f"h_all{i}", [128, 8 * B], BF16) for i in range(2)]
    h_new = [nc.alloc_sbuf_tensor(f"h_new{i}", [128, B], BF16) for i in range(2)]
    gact = [nc.alloc_sbuf_tensor(f"gact{i}", [128, 4 * B], F32) for i in range(2)]
    xp_t = nc.alloc_sbuf_tensor("xp_t", [128, 4 * 4 * B], F32)  # 4 prefetch bufs
    c_sb = nc.alloc_sbuf_tensor("c_sb", [128, B], F32)
    pad_sb = nc.alloc_sbuf_tensor("pad_sb", [128, 1], F32)
    tc_sb = nc.alloc_sbuf_tensor("tc_sb", [128, B], F32)
    t1_sb = nc.alloc_sbuf_tensor("t1_sb", [128, B], F32)

    # two 4-bank PSUM tensors (even/odd parity); bank g = cols [512g, 512g+512)
    ps = [nc.alloc_psum_tensor(f"ps{i}", [128, 2048], F32) for i in range(2)]

    # ---------------- semaphores ----------------
    load_sem = nc.alloc_semaphore("load_sem")
    init_sem = nc.alloc_semaphore("init_sem")
    gath_sems = [nc.alloc_semaphore(f"gath_sem{p}") for p in range(2)]
    pmm1_sem = nc.alloc_semaphore("pmm1_sem")
    pdve1_sem = nc.alloc_semaphore("pdve1_sem")
    xpout_sem = nc.alloc_semaphore("xpout_sem")
    xpin_sem = nc.alloc_semaphore("xpin_sem")
    mm_sem = nc.alloc_semaphore("mm_sem")
    gadd_sem = nc.alloc_semaphore("gadd_sem")
    act_sem = nc.alloc_semaphore("act_sem")
    dve2_sem = nc.alloc_semaphore("dve2_sem")
    act2_sem = nc.alloc_semaphore("act2_sem")
    hdone_sem = nc.alloc_semaphore("hdone_sem")
    dvx_sem = nc.alloc_semaphore("dvx_sem")
    hout_sem = nc.alloc_semaphore("hout_sem")
    prep_sem = nc.alloc_semaphore("prep_sem")
    loc_sem = nc.alloc_semaphore("loc_sem")
    rem_sems = [
        [nc.alloc_semaphore(f"rem_sem{s}_{p}") for p in range(2)]
        for s in range(N_CORES)
    ]

    nc.all_core_barrier()

    with nc.Block() as block:

        # ================= SYNC (SP): all HWDGE DMA =================
        @block.sync
        def _(sp):
            # initial loads
            sp.dma_start(out=xidx_sb[:, :], in_=xidx[:, :]).then_inc(load_sem, 16)
            sp.dma_start(out=wih_sb[:, :], in_=wih[:, :]).then_inc(load_sem, 16)
            sp.dma_start(out=whh_sb[:, :], in_=whh[:, :]).then_inc(load_sem, 16)
            sp.dma_start(out=bias_sb[:, :], in_=bias[:, :]).then_inc(load_sem, 16)

            # phase 1: write x_proj staging to HBM, group q covers t in
            # [16q, 16q+16), src xp_st[q%2] layout [p, (g, t16, b)]
            for q in range(N_QL):
                sp.wait_ge(pdve1_sem, q + 1)
                if q >= 1:
                    sp.wait_ge(xpout_sem, 16 * q)  # serialize: unmixed incs
                src = xp_st[q % 2][:, :].rearrange(
                    "p (g t b) -> p g t b", g=4, t=16, b=B
                )
                dst = xproj[:, :, 16 * q : 16 * (q + 1), :].rearrange(
                    "g p t b -> p g t b"
                )
                sp.dma_start(out=dst, in_=src).then_inc(xpout_sem, 16)

            # phase 2: x_proj per-step prefetch (2-step lead) + h writeback
            def prefetch(t):
                sp.wait_ge(xpout_sem, 16 * (t // 16 + 1))
                if t >= 1:
                    sp.wait_ge(xpin_sem, 16 * t)  # serialize: unmixed incs
                if t >= 4:
                    sp.wait_ge(gadd_sem, t - 3)  # buf WAR
                dst = xp_t[:, 128 * (t % 4) : 128 * (t % 4 + 1)].rearrange(
                    "p (g b) -> p g b", g=4, b=B
                )
                sp.dma_start(
                    out=dst, in_=xproj[:, :, t, :].rearrange("g p b -> p g b")
                ).then_inc(xpin_sem, 16)

            prefetch(0)
            if TT > 1:
                prefetch(1)
            for t in range(TT):
                if t + 2 < TT:
                    prefetch(t + 2)
                # h_{t+1} writeback
                sp.wait_ge(hdone_sem, t + 1)
                if t >= 1:
                    sp.wait_ge(hout_sem, 16 * t)  # serialize: unmixed incs
                sp.dma_start(out=hs_out[t, :, :], in_=h_new[t % 2][:, :]).then_inc(
                    hout_sem, 16
                )

            # final state
            sp.wait_ge(hdone_sem, TT)
            sp.wait_ge(hout_sem, 16 * TT)
            sp.dma_start(out=hf_out[:, :], in_=h_new[(TT - 1) % 2][:, :]).then_inc(
                hout_sem, 16
            )
            sp.wait_ge(hout_sem, 16 * (TT + 1))
            sp.dma_start(out=cf_out[:, :], in_=c_sb[:, :]).then_inc(hout_sem, 16)
            sp.wait_ge(hout_sem, 16 * (TT + 2))

        # ================= GPSIMD: gathers + broadcasts =================
        @block.gpsimd
        def _(gp):
            gp.load_library(library_config.mlp)
            gp.wait_ge(load_sem, 64)  # all initial loads
            for ci in range(N_CH):
                if ci >= 2:
                    gp.wait_ge(gath_sems[ci % 2], 16 * (ci // 2))  # unmixed incs
                    gp.wait_ge(pmm1_sem, (CH // TQ) * (ci - 1))  # embT buf WAR
                gp.dma_gather(
                    out_ap=embT[ci % 2][:, :, :],
                    in_ap=emb[:, :],
                    idxs_ap=xidx_sb[:, (CH // 16) * ci : (CH // 16) * (ci + 1)],
                    num_idxs=CH,
                    num_idxs_reg=CH,
                    elem_size=E,
                    transpose=True,
                ).then_inc(gath_sems[ci % 2], 16)

            for p in range(2):
                if N_CH > p:
                    gp.wait_ge(gath_sems[p], 16 * ((N_CH - p + 1) // 2))
            gp.load_library(library_config.remote_dma)

            pid = gp.to_reg(gp.partition_id())
            for t in range(TT):
                # reclaim SWDGE ring: at most 1 outstanding broadcast
                if t >= 1:
                    gp.wait_ge(loc_sem, 16 * t)
                for m in range(N_CORES):
                    with gp.If_cmp(pid, m, "IS_EQ"):
                        gp.remote_dma_broadcast(
                            out_ap=h_all[(t + 1) % 2][:, B * m : B * (m + 1)],
                            in_ap=h_new[t % 2][:, :],
                            remote_sem=rem_sems[m][(t + 1) % 2],
                            local_sem=loc_sem,
                            rdests=[(0, d) for d in range(N_CORES)],
                        ).then_inc(prep_sem, 1)
                gp.wait_ge(prep_sem, t + 1)
                gp.wait_ge(hdone_sem, t + 1)
                gp.trigger_dma(count=1)
            gp.wait_ge(loc_sem, 16 * TT)

        # ================= PE: matmuls =================
        @block.tensor
        def _(pe):
            pe.wait_ge(load_sem, 64)
            # phase 1: x_proj.T = W_ihT.T @ embT  per 512-token group
            for q in range(N_QL):
                ci, tc4 = q // (CH // TQ), q % (CH // TQ)
                par = q % 2
                pe.wait_ge(gath_sems[ci % 2], 16 * (ci // 2 + 1))
                if q >= 2:
                    pe.wait_ge(pdve1_sem, q - 1)  # psum bank WAR
                last = None
                for g in range(4):
                    for e in range(4):
                        last = pe.matmul(
                            ps[par][:, 512 * g : 512 * (g + 1)],
                            wih_sb[:, 128 * (4 * e + g) : 128 * (4 * e + g + 1)],
                            embT[ci % 2][:, e, TQ * tc4 : TQ * (tc4 + 1)],
                            start=(e == 0),
                            stop=(e == 3),
                        )
                last.then_inc(pmm1_sem, 1)

            # phase 2 recurrence
            pe.wait_ge(init_sem, 1)  # h_all[0], c zeroed
            pe.wait_ge(pdve1_sem, N_QL)  # phase-1 psum fully drained
            for t in range(TT):
                par = t % 2
                if t >= 2:
                    pe.wait_ge(gadd_sem, t - 1)  # psum bank WAR
                last = None
                # kc-outer: wait for each sender's chunk just before its MMs,
                # so PE compute overlaps the tail of the all-gather
                for kc in range(8):
                    if t >= 1:
                        pe.wait_ge(rem_sems[kc][t % 2], 2 * ((t + 1) // 2))
                    for g in range(4):
                        last = pe.matmul(
                            ps[par][:, 512 * g : 512 * g + B],
                            whh_sb[:, 128 * (4 * kc + g) : 128 * (4 * kc + g + 1)],
                            h_all[par][:, B * kc : B * (kc + 1)],
                            start=(kc == 0),
                            stop=(kc == 7),
                        )
                last.then_inc(mm_sem, 1)

        # ================= DVE =================
        @block.vector
        def _(v):
            # phase 1 evict: xp_st[q%2][:, g*TQ:(g+1)*TQ] = ps + bias[g]
            for q in range(N_QL):
                par = q % 2
                v.wait_ge(pmm1_sem, q + 1)
                if q >= 2:
                    v.wait_ge(xpout_sem, 16 * (q - 1))  # staging WAR
                last = None
                for g in range(4):
                    last = v.tensor_scalar_add(
                        xp_st[par][:, TQ * g : TQ * (g + 1)],
                        ps[par][:, 512 * g : 512 * (g + 1)],
                        bias_sb[:, g : g + 1],
                    )
                last.then_inc(pdve1_sem, 1)

            # phase 2 init
            v.memset(h_all[0][:, :], 0.0)
            v.memset(c_sb[:, :], 0.0).then_inc(init_sem, 1)

            for t in range(TT):
                par = t % 2
                # gates = psum + x_proj[t]
                v.wait_ge(mm_sem, t + 1)
                v.wait_ge(xpin_sem, 16 * (t + 1))
                if t >= 2:
                    v.wait_ge(act_sem, t - 1)  # gact WAR (ACT in-place reads)
                v.tensor_add(
                    gact[par][:, :].rearrange("p (g b) -> p g b", g=4, b=B),
                    ps[par][:, :].rearrange("p (g r) -> p g r", g=4, r=512)[
                        :, :, 0:B
                    ],
                    xp_t[:, 128 * (t % 4) : 128 * (t % 4 + 1)].rearrange(
                        "p (g b) -> p g b", g=4, b=B
                    ),
                ).then_inc(gadd_sem, 1)

                # c = f*c + i*g ; col layout: i|f|o|g
                # (ordering + filler keeps every same-engine RAW >= 2 slots)
                v.wait_ge(act_sem, t + 1)
                v.tensor_mul(c_sb[:, :], c_sb[:, :], gact[par][:, B : 2 * B])
                v.tensor_mul(
                    t1_sb[:, :], gact[par][:, 0:B], gact[par][:, 3 * B : 4 * B]
                ).then_inc(dvx_sem, 1)
                v.wait_ge(dvx_sem, t + 1)
                v.tensor_add(c_sb[:, :], c_sb[:, :], t1_sb[:, :]).then_inc(
                    dve2_sem, 1
                )

                # h = o * tanh(c)   (bf16 out)
                v.wait_ge(act2_sem, t + 1)
                if t >= 2:
                    v.wait_ge(hout_sem, 16 * (t - 1))  # h_new WAR vs writeback
                    v.wait_ge(loc_sem, 16 * (t - 1))  # h_new WAR vs broadcast
                v.tensor_mul(
                    h_new[par][:, :], gact[par][:, 2 * B : 3 * B], tc_sb[:, :]
                ).then_inc(hdone_sem, 1)

        # ================= ACT =================
        @block.scalar
        def _(a):
            for t in range(TT):
                par = t % 2
                a.wait_ge(gadd_sem, t + 1)
                if t >= 1:
                    a.wait_ge(hdone_sem, t)  # tc WAR + gact read-done
                a.activation(
                    gact[par][:, 0 : 3 * B], gact[par][:, 0 : 3 * B], AF.Sigmoid
                )
                a.activation(
                    gact[par][:, 3 * B : 4 * B], gact[par][:, 3 * B : 4 * B], AF.Tanh
                ).then_inc(act_sem, 1)

                a.wait_ge(dve2_sem, t + 1)
                a.activation(tc_sb[:, :], c_sb[:, :], AF.Tanh).then_inc(act2_sem, 1)

    lower_extended_insts(nc)
    return nc


# ======================================================================
# host-side prep
# ======================================================================


def _gate_rows(k):
    """Rows of the 4H gate dim owned by core k, in i|f|o|g chunk order."""
    r = np.arange(128) + 128 * k
    return np.concatenate([r, H + r, 3 * H + r, 2 * H + r])  # i, f, o, g


def _prep_core_inputs(x, emb_bf16, W_ih, W_hh, b_sum):
    """Per-core input dicts (host sharding)."""
    tokl = x.size
    # token indices, t-major, wrapped in 16 partitions, replicated to 128
    tok = x.T.reshape(-1).astype(np.int16)  # [T*B] t-major
    idx16 = tok.reshape(tokl // 16, 16).T  # [16, TOK/16]
    idx128 = np.tile(idx16, (8, 1))  # [128, TOK/16]

    ins = []
    for k in range(N_CORES):
        rows = _gate_rows(k)
        wih_k = W_ih[rows, :].T  # [E, G]
        wih_k = (
            wih_k.reshape(4, 128, 4, 128)
            .transpose(1, 0, 2, 3)
            .reshape(128, 16 * 128)
            .astype(ml_dtypes.bfloat16)
        )
        whh_k = W_hh[rows, :].T  # [H, G]
        whh_k = (
            whh_k.reshape(8, 128, 4, 128)
            .transpose(1, 0, 2, 3)
            .reshape(128, 32 * 128)
            .astype(ml_dtypes.bfloat16)
        )
        bias_k = b_sum[rows].reshape(4, 128).T.astype(np.float32).copy()  # [128,4]
        ins.append(
            {
                "emb": emb_bf16,
                "xidx": idx128,
                "wih": wih_k,
                "whh": whh_k,
                "bias": bias_k,
            }
        )
    return ins


def kernel(x, emb_table, W_ih, W_hh, b_ih, b_hh):
    x = np.asarray(x)
    emb_table = np.asarray(emb_table, np.float32)
    W_ih = np.asarray(W_ih, np.float32)
    W_hh = np.asarray(W_hh, np.float32)
    b_sum = np.asarray(b_ih, np.float32) + np.asarray(b_hh, np.float32)

    emb_bf16 = emb_table.astype(ml_dtypes.bfloat16)
    in_maps = _prep_core_inputs(x, emb_bf16, W_ih, W_hh, b_sum)

    t_steps = x.shape[1]
    if ("nc", t_steps) not in _cache:
        _cache[("nc", t_steps)] = build_nc(t_steps)
    nc = _cache[("nc", t_steps)]

    from concourse.bass_utils import run_bass_kernel_spmd

    res = run_bass_kernel_spmd(
        nc, in_maps, core_ids=list(range(N_CORES)), **_cache.get("run_kwargs", {})
    )
    _cache["last_res"] = res

    hs = np.stack(
        [
            np.asarray(r["hs"])
            .view(ml_dtypes.bfloat16)
            .astype(np.float32)
            .reshape(t_steps, 128, B)
            for r in res.results
        ]
    )  # [8, T, 128, B]
    enc = np.ascontiguousarray(hs.transpose(3, 1, 0, 2)).reshape(B, t_steps, H)
    hf = np.stack(
        [
            np.asarray(r["hf"])
            .view(ml_dtypes.bfloat16)
            .astype(np.float32)
            .reshape(128, B)
            for r in res.results
        ]
    )  # [8, 128, B]
    cf = np.stack(
        [np.asarray(r["cf"]).reshape(128, B) for r in res.results]
    )  # [8, 128, B]
    state_h = np.ascontiguousarray(hf.transpose(2, 0, 1)).reshape(B, H)
    state_c = np.ascontiguousarray(cf.transpose(2, 0, 1)).reshape(B, H)
    return enc, state_h, state_c


# revision 23
# speedup vs baseline: 1.2188x; 1.2188x over previous
"""LSTM encoder (nn_Encoder_83528523972689) on 8 TRN2 NeuronCores.

Strategy: tensor-parallel split of the 4H gate dimension across 8 cores.
Each core owns a 128-wide slice of H for every gate (i/f/o/g). Per step it
computes its gates.T [512, 32] slice (W_hh slice stationary in the PE,
h.T moving), updates its c/h slices, and broadcasts its h.T chunk
[128, 32] to all 8 cores via remote_dma_broadcast (direct SBUF->SBUF,
relative XOR routing). x_proj = emb[x] @ W_ih.T + b is precomputed in a
parallel phase (embedding rows gathered transposed via dma_gather).

Everything is raw Bass (no Tile), fully unrolled over the 512 steps,
monotone semaphore thresholds, with a startup all-core barrier.
"""

import sys

sys.path.insert(0, "/opt/trn_rl_repo")

import numpy as np
import ml_dtypes

import concourse.bass as bass
import concourse.mybir as mybir
from concourse import library_config
from concourse.library_overlay import lower_extended_insts

F32 = mybir.dt.float32
BF16 = mybir.dt.bfloat16
I16 = mybir.dt.int16
AF = mybir.ActivationFunctionType

N_CORES = 8
B = 32  # batch
T = 512  # seq len
E = 512  # embedding dim
H = 1024  # hidden
G = 512  # gate rows per core (4 gates x 128)
VOCAB = 32000

# tokens processed per phase-1 gather chunk
TOK = B * T  # 16384
CHUNK = 2048
N_CHUNK = TOK // CHUNK  # 8
TQ = 512  # tokens per phase-1 matmul group
N_Q = TOK // TQ  # 32 groups

_cache = {}


def build_nc(t_steps=T):
    TT = t_steps
    TOKL = B * TT
    CH = min(512, TOKL)
    N_CH = TOKL // CH
    N_QL = TOKL // TQ
    nc = bass.Bass(num_devices=N_CORES)

    # ---------------- I/O ----------------
    emb = nc.declare_dram_parameter("emb", [VOCAB, E], BF16, isOutput=False)
    xidx = nc.declare_dram_parameter("xidx", [128, TOKL // 16], I16, isOutput=False)
    wih = nc.declare_dram_parameter("wih", [128, 16 * 128], BF16, isOutput=False)
    whh = nc.declare_dram_parameter("whh", [128, 32 * 128], BF16, isOutput=False)
    bias = nc.declare_dram_parameter("bias", [128, 4], F32, isOutput=False)
    ident = nc.declare_dram_parameter("ident", [128, 128], BF16, isOutput=False)
    hs_out = nc.declare_dram_parameter("hs", [TT, 128, B], BF16, isOutput=True)
    hf_out = nc.declare_dram_parameter("hf", [128, B], BF16, isOutput=True)
    cf_out = nc.declare_dram_parameter("cf", [128, B], F32, isOutput=True)

    # internal HBM scratch: x_proj.T slice, layout [4 g][128 p][T t][B b] bf16
    xproj = nc.dram_tensor("xproj", [4, 128, TT, B], BF16)

    # ---------------- SBUF ----------------
    xidx_sb = nc.alloc_sbuf_tensor("xidx_sb", [128, TOKL // 16], I16)
    wih_sb = nc.alloc_sbuf_tensor("wih_sb", [128, 16 * 128], BF16)
    whh_sb = nc.alloc_sbuf_tensor("whh_sb", [128, 32 * 128], BF16)
    bias_sb = nc.alloc_sbuf_tensor("bias_sb", [128, 4], F32)
    ident_sb = nc.alloc_sbuf_tensor("ident_sb", [128, 128], BF16)
    embT = [
        nc.alloc_sbuf_tensor(f"embT{i}", [128, 4, CH], BF16) for i in range(2)
    ]
    xp_st = [
        nc.alloc_sbuf_tensor(f"xp_st{i}", [128, 4 * TQ], BF16) for i in range(2)
    ]
    # phase 2
    h_all = [nc.alloc_sbuf_tensor(f"h_all{i}", [128, 8 * B], BF16) for i in range(2)]
    h_new = [nc.alloc_sbuf_tensor(f"h_new{i}", [128, B], BF16) for i in range(2)]
    gact = [nc.alloc_sbuf_tensor(f"gact{i}", [128, 4 * B], F32) for i in range(2)]
    xp_t = nc.alloc_sbuf_tensor("xp_t", [128, 4 * 4 * B], BF16)  # 4 prefetch bufs
    c_sb = nc.alloc_sbuf_tensor("c_sb", [128, B], F32)
    pad_sb = nc.alloc_sbuf_tensor("pad_sb", [128, 1], F32)
    tc_sb = nc.alloc_sbuf_tensor("tc_sb", [128, B], F32)
    t1_sb = nc.alloc_sbuf_tensor("t1_sb", [128, B], F32)

    # two 4-bank PSUM tensors (even/odd parity); bank g = cols [512g, 512g+512)
    ps = [nc.alloc_psum_tensor(f"ps{i}", [128, 2048], F32) for i in range(2)]

    # ---------------- semaphores ----------------
    load_sem = nc.alloc_semaphore("load_sem")
    init_sem = nc.alloc_semaphore("init_sem")
    gath_sems = [nc.alloc_semaphore(f"gath_sem{p}") for p in range(2)]
    pmm1_sem = nc.alloc_semaphore("pmm1_sem")
    pdve1_sem = nc.alloc_semaphore("pdve1_sem")
    xpout_sem = nc.alloc_semaphore("xpout_sem")
    xpin_sem = nc.alloc_semaphore("xpin_sem")
    mm_sem = nc.alloc_semaphore("mm_sem")
    xprd_sem = nc.alloc_semaphore("xprd_sem")
    act_sem = nc.alloc_semaphore("act_sem")
    actg_sem = nc.alloc_semaphore("actg_sem")
    dve2_sem = nc.alloc_semaphore("dve2_sem")
    act2_sem = nc.alloc_semaphore("act2_sem")
    hdone_sem = nc.alloc_semaphore("hdone_sem")
    dvx_sem = nc.alloc_semaphore("dvx_sem")
    hout_sem = nc.alloc_semaphore("hout_sem")
    prep_sem = nc.alloc_semaphore("prep_sem")
    loc_sem = nc.alloc_semaphore("loc_sem")
    rem_sems = [
        [nc.alloc_semaphore(f"rem_sem{s}_{p}") for p in range(2)]
        for s in range(N_CORES)
    ]

    # startup cross-core barrier, overlapped with phase 1: the AllReduce
    # runs on TOPSP/SDMA while the engines do local work; only the first
    # h-broadcast (first cross-core effect) gates on its completion.
    bar_buf = nc.dram_tensor("bar_buf", [128, 1], F32)
    bar_sem = nc.alloc_semaphore("bar_sem")

    with nc.Block() as block:

        # ================= SYNC (SP): all HWDGE DMA =================
        @block.sync
        def _(sp):
            # initial loads
            sp.wait_ge(init_sem, 1)  # pad_sb zeroed for barrier seed
            sp.dma_start(out=bar_buf[:, :], in_=pad_sb[:, :]).then_inc(
                bar_sem, 16
            )
            sp.dma_start(out=xidx_sb[:, :], in_=xidx[:, :]).then_inc(load_sem, 16)
            sp.dma_start(out=wih_sb[:, :], in_=wih[:, :]).then_inc(load_sem, 16)
            sp.dma_start(out=whh_sb[:, :], in_=whh[:, :]).then_inc(load_sem, 16)
            sp.dma_start(out=bias_sb[:, :], in_=bias[:, :]).then_inc(load_sem, 16)
            sp.dma_start(out=ident_sb[:, :], in_=ident[:, :]).then_inc(load_sem, 16)

            # phase 1: write x_proj staging to HBM, group q covers t in
            # [16q, 16q+16), src xp_st[q%2] layout [p, (g, t16, b)]
            for q in range(N_QL):
                sp.wait_ge(pdve1_sem, q + 1)
                if q >= 1:
                    sp.wait_ge(xpout_sem, 16 * q)  # serialize: unmixed incs
                src = xp_st[q % 2][:, :].rearrange(
                    "p (g t b) -> p g t b", g=4, t=16, b=B
                )
                dst = xproj[:, :, 16 * q : 16 * (q + 1), :].rearrange(
                    "g p t b -> p g t b"
                )
                sp.dma_start(out=dst, in_=src).then_inc(xpout_sem, 16)

            # phase 2: x_proj per-step prefetch (2-step lead) + h writeback
            def prefetch(t):
                sp.wait_ge(xpout_sem, 16 * (t // 16 + 1))
                if t >= 1:
                    sp.wait_ge(xpin_sem, 16 * t)  # serialize: unmixed incs
                if t >= 4:
                    sp.wait_ge(xprd_sem, t - 3)  # buf WAR (PE id-MM consumed)
                dst = xp_t[:, 128 * (t % 4) : 128 * (t % 4 + 1)].rearrange(
                    "p (g b) -> p g b", g=4, b=B
                )
                sp.dma_start(
                    out=dst, in_=xproj[:, :, t, :].rearrange("g p b -> p g b")
                ).then_inc(xpin_sem, 16)

            prefetch(0)
            if TT > 1:
                prefetch(1)
            for t in range(TT):
                if t + 2 < TT:
                    prefetch(t + 2)
                # h_{t+1} writeback
                sp.wait_ge(hdone_sem, t + 1)
                if t >= 1:
                    sp.wait_ge(hout_sem, 16 * t)  # serialize: unmixed incs
                sp.dma_start(out=hs_out[t, :, :], in_=h_new[t % 2][:, :]).then_inc(
                    hout_sem, 16
                )

            # final state
            sp.wait_ge(hdone_sem, TT)
            sp.wait_ge(hout_sem, 16 * TT)
            sp.dma_start(out=hf_out[:, :], in_=h_new[(TT - 1) % 2][:, :]).then_inc(
                hout_sem, 16
            )
            sp.wait_ge(hout_sem, 16 * (TT + 1))
            sp.dma_start(out=cf_out[:, :], in_=c_sb[:, :]).then_inc(hout_sem, 16)
            sp.wait_ge(hout_sem, 16 * (TT + 2))

        # ================= GPSIMD: gathers + broadcasts =================
        @block.gpsimd
        def _(gp):
            gp.wait_ge(bar_sem, 16)
            gp.collective_compute(
                "AllReduce",
                mybir.AluOpType.add,
                replica_groups=[list(range(N_CORES))],
                ins=[bar_buf[:, :].opt()],
                outs=[bar_buf[:, :].opt()],
            ).then_inc(bar_sem, 1)
            gp.load_library(library_config.mlp)
            gp.wait_ge(load_sem, 80)  # all initial loads
            for ci in range(N_CH):
                if ci >= 2:
                    gp.wait_ge(gath_sems[ci % 2], 16 * (ci // 2))  # unmixed incs
                    gp.wait_ge(pmm1_sem, (CH // TQ) * (ci - 1))  # embT buf WAR
                gp.dma_gather(
                    out_ap=embT[ci % 2][:, :, :],
                    in_ap=emb[:, :],
                    idxs_ap=xidx_sb[:, (CH // 16) * ci : (CH // 16) * (ci + 1)],
                    num_idxs=CH,
                    num_idxs_reg=CH,
                    elem_size=E,
                    transpose=True,
                ).then_inc(gath_sems[ci % 2], 16)

            for p in range(2):
                if N_CH > p:
                    gp.wait_ge(gath_sems[p], 16 * ((N_CH - p + 1) // 2))
            gp.load_library(library_config.remote_dma)

            gp.wait_ge(bar_sem, 17)  # all cores started (barrier done)
            pid = gp.to_reg(gp.partition_id())
            for t in range(TT):
                # reclaim SWDGE ring: at most 1 outstanding broadcast
                if t >= 1:
                    gp.wait_ge(loc_sem, 16 * t)
                for m in range(N_CORES):
                    with gp.If_cmp(pid, m, "IS_EQ"):
                        gp.remote_dma_broadcast(
                            out_ap=h_all[(t + 1) % 2][:, B * m : B * (m + 1)],
                            in_ap=h_new[t % 2][:, :],
                            remote_sem=rem_sems[m][(t + 1) % 2],
                            local_sem=loc_sem,
                            rdests=[(0, d) for d in range(N_CORES)],
                        ).then_inc(prep_sem, 1)
                gp.wait_ge(prep_sem, t + 1)
                gp.wait_ge(hdone_sem, t + 1)
                gp.trigger_dma(count=1)
            gp.wait_ge(loc_sem, 16 * TT)

        # ================= PE: matmuls =================
        @block.tensor
        def _(pe):
            pe.wait_ge(load_sem, 80)
            # phase 1: x_proj.T = W_ihT.T @ embT  per 512-token group
            for q in range(N_QL):
                ci, tc4 = q // (CH // TQ), q % (CH // TQ)
                par = q % 2
                pe.wait_ge(gath_sems[ci % 2], 16 * (ci // 2 + 1))
                if q >= 2:
                    pe.wait_ge(pdve1_sem, q - 1)  # psum bank WAR
                last = None
                for g in range(4):
                    for e in range(4):
                        last = pe.matmul(
                            ps[par][:, 512 * g : 512 * (g + 1)],
                            wih_sb[:, 128 * (4 * e + g) : 128 * (4 * e + g + 1)],
                            embT[ci % 2][:, e, TQ * tc4 : TQ * (tc4 + 1)],
                            start=(e == 0),
                            stop=(e == 3),
                        )
                last.then_inc(pmm1_sem, 1)

            # phase 2 recurrence
            pe.wait_ge(init_sem, 2)  # h_all[0], c zeroed
            for t in range(TT):
                if t < 2:
                    # phase-1 drain of this parity's banks
                    pe.wait_ge(pdve1_sem, max(0, N_QL - 1 + t))
                par = t % 2
                if t >= 2:
                    pe.wait_ge(actg_sem, t - 1)  # psum bank WAR (ACT read done)
                pe.wait_ge(xpin_sem, 16 * (t + 1))  # x_proj[t] prefetched
                # inject x_proj into PSUM (identity matmul) before the h MMs
                last = None
                for g in range(4):
                    last = pe.matmul(
                        ps[par][:, 512 * g : 512 * g + B],
                        ident_sb[:, :],
                        xp_t[:, 128 * (t % 4) + B * g : 128 * (t % 4) + B * (g + 1)],
                        start=True,
                        stop=False,
                    )
                last.then_inc(xprd_sem, 1)
                last = None
                # kc-outer: wait for each sender's chunk just before its MMs,
                # so PE compute overlaps the tail of the all-gather
                for kc in range(8):
                    if t >= 1:
                        pe.wait_ge(rem_sems[kc][t % 2], 2 * ((t + 1) // 2))
                    for g in range(4):
                        last = pe.matmul(
                            ps[par][:, 512 * g : 512 * g + B],
                            whh_sb[:, 128 * (4 * kc + g) : 128 * (4 * kc + g + 1)],
                            h_all[par][:, B * kc : B * (kc + 1)],
                            start=False,
                            stop=(kc == 7),
                        )
                last.then_inc(mm_sem, 1)

        # ================= DVE =================
        @block.vector
        def _(v):
            v.memset(pad_sb[:, :], 0.0).then_inc(init_sem, 1)  # barrier seed
            # phase 1 evict: xp_st[q%2][:, g*TQ:(g+1)*TQ] = ps + bias[g]
            for q in range(N_QL):
                par = q % 2
                v.wait_ge(pmm1_sem, q + 1)
                if q >= 2:
                    v.wait_ge(xpout_sem, 16 * (q - 1))  # staging WAR
                last = None
                for g in range(4):
                    last = v.tensor_scalar_add(
                        xp_st[par][:, TQ * g : TQ * (g + 1)],
                        ps[par][:, 512 * g : 512 * (g + 1)],
                        bias_sb[:, g : g + 1],
                    )
                last.then_inc(pdve1_sem, 1)

            # phase 2 init
            v.memset(h_all[0][:, :], 0.0)
            v.memset(c_sb[:, :], 0.0).then_inc(init_sem, 1)  # -> init_sem == 2

            for t in range(TT):
                par = t % 2
                # c = f*c + i*g ; col layout: i|f|o|g
                # c-mul overlaps ACT's tanh(g) (split act semaphores)
                v.wait_ge(act_sem, t + 1)
                v.tensor_mul(c_sb[:, :], c_sb[:, :], gact[par][:, B : 2 * B])
                v.wait_ge(actg_sem, t + 1)
                v.tensor_mul(
                    t1_sb[:, :], gact[par][:, 0:B], gact[par][:, 3 * B : 4 * B]
                ).then_inc(dvx_sem, 1)
                v.wait_ge(dvx_sem, t + 1)
                v.tensor_add(c_sb[:, :], c_sb[:, :], t1_sb[:, :]).then_inc(
                    dve2_sem, 1
                )

                # h = o * tanh(c)   (bf16 out)
                v.wait_ge(act2_sem, t + 1)
                if t >= 2:
                    v.wait_ge(hout_sem, 16 * (t - 1))  # h_new WAR vs writeback
                    v.wait_ge(loc_sem, 16 * (t - 1))  # h_new WAR vs broadcast
                v.tensor_mul(
                    h_new[par][:, :], gact[par][:, 2 * B : 3 * B], tc_sb[:, :]
                ).then_inc(hdone_sem, 1)

        # ================= ACT =================
        @block.scalar
        def _(a):
            a.wait_ge(init_sem, 1)
            a.activation(tc_sb[:, 0:1], pad_sb[:, :], AF.Sigmoid)  # LUT prewarm
            a.activation(tc_sb[:, 1:2], pad_sb[:, :], AF.Tanh)
            for t in range(TT):
                par = t % 2
                a.wait_ge(mm_sem, t + 1)
                if t >= 1:
                    a.wait_ge(hdone_sem, t)  # tc WAR + gact read-done
                psv = ps[par][:, :].rearrange("p (g r) -> p g r", g=4, r=512)
                a.activation(
                    gact[par][:, 0 : 3 * B].rearrange("p (g b) -> p g b", g=3, b=B),
                    psv[:, 0:3, 0:B],
                    AF.Sigmoid,
                ).then_inc(act_sem, 1)
                a.activation(
                    gact[par][:, 3 * B : 4 * B], psv[:, 3, 0:B], AF.Tanh
                ).then_inc(actg_sem, 1)

                a.wait_ge(dve2_sem, t + 1)
                a.activation(tc_sb[:, :], c_sb[:, :], AF.Tanh).then_inc(act2_sem, 1)

    lower_extended_insts(nc)
    return nc


# ======================================================================
# host-side prep
# ======================================================================


def _gate_rows(k):
    """Rows of the 4H gate dim owned by core k, in i|f|o|g chunk order."""
    r = np.arange(128) + 128 * k
    return np.concatenate([r, H + r, 3 * H + r, 2 * H + r])  # i, f, o, g


def _prep_core_inputs(x, emb_bf16, W_ih, W_hh, b_sum):
    """Per-core input dicts (host sharding)."""
    tokl = x.size
    # token indices, t-major, wrapped in 16 partitions, replicated to 128
    tok = x.T.reshape(-1).astype(np.int16)  # [T*B] t-major
    idx16 = tok.reshape(tokl // 16, 16).T  # [16, TOK/16]
    idx128 = np.tile(idx16, (8, 1))  # [128, TOK/16]

    ins = []
    for k in range(N_CORES):
        rows = _gate_rows(k)
        wih_k = W_ih[rows, :].T  # [E, G]
        wih_k = (
            wih_k.reshape(4, 128, 4, 128)
            .transpose(1, 0, 2, 3)
            .reshape(128, 16 * 128)
            .astype(ml_dtypes.bfloat16)
        )
        whh_k = W_hh[rows, :].T  # [H, G]
        whh_k = (
            whh_k.reshape(8, 128, 4, 128)
            .transpose(1, 0, 2, 3)
            .reshape(128, 32 * 128)
            .astype(ml_dtypes.bfloat16)
        )
        bias_k = b_sum[rows].reshape(4, 128).T.astype(np.float32).copy()  # [128,4]
        ins.append(
            {
                "emb": emb_bf16,
                "xidx": idx128,
                "wih": wih_k,
                "whh": whh_k,
                "bias": bias_k,
                "ident": np.eye(128, dtype=ml_dtypes.bfloat16),
            }
        )
    return ins


def kernel(x, emb_table, W_ih, W_hh, b_ih, b_hh):
    x = np.asarray(x)
    emb_table = np.asarray(emb_table, np.float32)
    W_ih = np.asarray(W_ih, np.float32)
    W_hh = np.asarray(W_hh, np.float32)
    b_sum = np.asarray(b_ih, np.float32) + np.asarray(b_hh, np.float32)

    emb_bf16 = emb_table.astype(ml_dtypes.bfloat16)
    in_maps = _prep_core_inputs(x, emb_bf16, W_ih, W_hh, b_sum)

    t_steps = x.shape[1]
    if ("nc", t_steps) not in _cache:
        _cache[("nc", t_steps)] = build_nc(t_steps)
    nc = _cache[("nc", t_steps)]

    from concourse.bass_utils import run_bass_kernel_spmd

    try:
        res = run_bass_kernel_spmd(
            nc, in_maps, core_ids=list(range(N_CORES)), **_cache.get("run_kwargs", {})
        )
    except Exception:
        # transient NRT/tunnel failures have been observed; retry once
        res = run_bass_kernel_spmd(
            nc, in_maps, core_ids=list(range(N_CORES)), **_cache.get("run_kwargs", {})
        )
    _cache["last_res"] = res

    hs = np.stack(
        [
            np.asarray(r["hs"])
            .view(ml_dtypes.bfloat16)
            .astype(np.float32)
            .reshape(t_steps, 128, B)
            for r in res.results
        ]
    )  # [8, T, 128, B]
    enc = np.ascontiguousarray(hs.transpose(3, 1, 0, 2)).reshape(B, t_steps, H)
    hf = np.stack(
        [
            np.asarray(r["hf"])
            .view(ml_dtypes.bfloat16)
            .astype(np.float32)
            .reshape(128, B)
            for r in res.results
        ]
    )  # [8, 128, B]
    cf = np.stack(
        [np.asarray(r["cf"]).reshape(128, B) for r in res.results]
    )  # [8, 128, B]
    state_h = np.ascontiguousarray(hf.transpose(2, 0, 1)).reshape(B, H)
    state_c = np.ascontiguousarray(cf.transpose(2, 0, 1)).reshape(B, H)
    return enc, state_h, state_c
